# revision 2
# baseline (speedup 1.0000x reference)
"""Multi-headed self-attention (B=64, S=512, E=1024, H=16, causal, no 1/sqrt(d)
scale) as a Bass/Tile kernel for 8 Trainium2 NeuronCores.

Sharding: data-parallel over batch — each core processes 8 batches with
replicated weights; no collectives. Matmuls run in fp16 (projections, scores,
out-proj) / bf16 (attention*V, needed for exp() range) with fp32 PSUM
accumulation. Softmax skips the max-subtraction: scores for this problem are
bounded (|s| < 90 ⇒ exp() stays finite in fp32), and the denominators are
produced by augmenting V with a ones-column so the AV matmul emits them
directly.

Self-contained: hardcodes shapes; only needs concourse (on sys.path in the
container) + numpy.
"""

import numpy as np
from contextlib import ExitStack

import concourse.bass as bass
import concourse.tile as tile
from concourse import bacc, mybir
from concourse.bass_interp import get_hw_module
from concourse.bass_utils import run_bass_kernel_spmd
from concourse.masks import make_identity

F32 = mybir.dt.float32
F16 = mybir.dt.float16
BF16 = mybir.dt.bfloat16

B, S, E, H, D = 64, 512, 1024, 16, 64
N_CORES = 8
BL = B // N_CORES            # batches per core
TOK = BL * S                 # tokens per core
KE = E // 128                # 128-row tiles along e (8)
NT = S // 128                # 128-token tiles per batch (4)
NEG = -1.0e30


def build_module():
    nc = bacc.Bacc("TRN2", target_bir_lowering=False, debug=False,
                   num_devices=N_CORES)
    x_ap = nc.dram_tensor("x", [TOK, E], F32, kind="ExternalInput").ap()
    w_aps = {
        name: nc.dram_tensor(name, [E, E], F32, kind="ExternalInput").ap()
        for name in ("wq", "wk", "wv", "wo")
    }
    bo_ap = nc.dram_tensor("bo", [E], F32, kind="ExternalInput").ap()
    y_ap = nc.dram_tensor("y", [TOK, E], F32, kind="ExternalOutput").ap()

    with tile.TileContext(nc) as tc, ExitStack() as ctx:
        consts = ctx.enter_context(tc.tile_pool(name="consts", bufs=1))
        stage = ctx.enter_context(tc.tile_pool(name="stage", bufs=2))
        bigs = ctx.enter_context(tc.tile_pool(name="bigs", bufs=2))
        ppool = ctx.enter_context(tc.tile_pool(name="ppool", bufs=2))
        ypool = ctx.enter_context(tc.tile_pool(name="ypool", bufs=2))
        small = ctx.enter_context(tc.tile_pool(name="small", bufs=4))
        psum = ctx.enter_context(tc.tile_pool(name="psum", bufs=6, space="PSUM"))
        psum_o = ctx.enter_context(tc.tile_pool(name="psum_o", bufs=2, space="PSUM"))

        # ---- constants ----
        ident = consts.tile([128, 128], F16)
        make_identity(nc, ident[:])
        # additive causal mask for the diagonal 128x128 block of S^T[k, q]:
        # keep 0 where k <= q, else -1e30
        mask = consts.tile([128, 128], F32)
        nc.gpsimd.memset(mask[:], 0.0)
        nc.gpsimd.affine_select(
            out=mask[:], in_=mask[:], compare_op=mybir.AluOpType.is_ge,
            fill=NEG, base=0, channel_multiplier=-1, pattern=[[1, 128]],
        )
        ones_r = consts.tile([1, 128], F16)
        nc.vector.memset(ones_r[:], 1.0)
        bo_st = stage.tile([1, E], F32, tag="wstage")
        nc.sync.dma_start(bo_st[:], bo_ap[None, :])
        bo_sb = consts.tile([1, E], F16)
        nc.vector.tensor_copy(bo_sb[:], bo_st[:])

        w_sb = {}
        for name in ("wq", "wk", "wv", "wo"):
            wt = consts.tile([128, KE, E], F16, tag=name)
            for k in range(KE):
                ws = stage.tile([128, E], F32, tag="wstage")
                nc.sync.dma_start(ws[:], w_aps[name][k * 128:(k + 1) * 128, :])
                nc.vector.tensor_copy(wt[:, k, :], ws[:])
            w_sb[name] = wt

        # ---- per batch ----
        for b in range(BL):
            r0 = b * S
            # 1) load X, cast fp16, transpose via PE (identity matmul)
            xT = bigs.tile([128, KE, S], F16, tag="xT")   # [e, tok] fp16
            for t in range(NT):
                xs = stage.tile([128, E], F32, tag="xstage")
                nc.sync.dma_start(xs[:], x_ap[r0 + t * 128: r0 + (t + 1) * 128, :])
                xf = stage.tile([128, E], F16, tag="xf16")
                nc.vector.tensor_copy(xf[:], xs[:])
                for e in range(KE):
                    ptr = psum.tile([128, 128], F32, tag="mm512")
                    nc.tensor.matmul(ptr[:], lhsT=xf[:, e * 128:(e + 1) * 128],
                                     rhs=ident[:], start=True, stop=True)
                    nc.vector.tensor_copy(xT[:, e, t * 128:(t + 1) * 128], ptr[:])

            # 2) Q^T, K^T : [e_out, tok] fp16
            qT = bigs.tile([128, KE, S], F16, tag="qT")
            kT = bigs.tile([128, KE, S], F16, tag="kT")
            for eo in range(KE):
                for wname, dst in (("wq", qT), ("wk", kT)):
                    ps = psum.tile([128, S], F32, tag="mm512")
                    for k in range(KE):
                        nc.tensor.matmul(
                            ps[:], lhsT=w_sb[wname][:, k, eo * 128:(eo + 1) * 128],
                            rhs=xT[:, k, :], start=(k == 0), stop=(k == KE - 1))
                    nc.vector.tensor_copy(dst[:, eo, :], ps[:])

            # 3) V natural [tok, h, d] + ones column, bf16
            v_sb = bigs.tile([128, NT, H, D + 1], BF16, tag="v")
            for t in range(NT):
                for c in range(2):
                    ps = psum.tile([128, S], F32, tag="mm512")
                    for k in range(KE):
                        nc.tensor.matmul(
                            ps[:], lhsT=xT[:, k, t * 128:(t + 1) * 128],
                            rhs=w_sb["wv"][:, k, c * 512:(c + 1) * 512],
                            start=(k == 0), stop=(k == KE - 1))
                    nc.vector.tensor_copy(
                        v_sb[:, t, c * 8:(c + 1) * 8, 0:D],
                        ps[:].rearrange("p (h d) -> p h d", h=8))
                nc.vector.memset(v_sb[:, t, :, D:D + 1], 1.0)

            # 4) attention per head; S^T[k, q] layout, ragged causal
            oT = bigs.tile([128, KE, S], F16, tag="oT")
            for h in range(H):
                j, p0 = h // 2, 64 * (h % 2)
                pt = ppool.tile([128, NT, S], BF16, tag="pT")
                for i in range(NT):
                    w0 = i * 128
                    ps = psum.tile([128, S], F32, tag="mm512")
                    nc.tensor.matmul(
                        ps[:, w0:S], lhsT=kT[p0:p0 + 64, j, w0:w0 + 128],
                        rhs=qT[p0:p0 + 64, j, w0:S], start=True, stop=True)
                    nc.vector.tensor_add(ps[:, w0:w0 + 128], ps[:, w0:w0 + 128],
                                         mask[:])
                    nc.scalar.activation(pt[:, i, w0:S], ps[:, w0:S],
                                         mybir.ActivationFunctionType.Exp)
                po = psum_o.tile([D + 1, S], F32, tag="ps_o")
                for i in range(NT):
                    w0 = i * 128
                    nc.tensor.matmul(po[:, w0:S], lhsT=v_sb[:, i, h, :],
                                     rhs=pt[:, i, w0:S],
                                     start=(i == 0), stop=(i == NT - 1))
                linv = small.tile([1, S], F32, tag="linv")
                nc.vector.reciprocal(linv[:], po[D:D + 1, :])
                linb = small.tile([64, S], F32, tag="linb")
                nc.gpsimd.partition_broadcast(linb[:], linv[:])
                nc.vector.tensor_mul(oT[p0:p0 + 64, j, :], po[0:D, :], linb[:])

            # 5) out-projection + bias, write y
            for t in range(NT):
                for c in range(2):
                    ps = psum.tile([128, S], F32, tag="mm512")
                    for k in range(KE):
                        nc.tensor.matmul(
                            ps[:], lhsT=oT[:, k, t * 128:(t + 1) * 128],
                            rhs=w_sb["wo"][:, k, c * 512:(c + 1) * 512],
                            start=(k == 0), stop=False)
                    nc.tensor.matmul(ps[:], lhsT=ones_r[:],
                                     rhs=bo_sb[:, c * 512:(c + 1) * 512],
                                     start=False, stop=True)
                    yc = ypool.tile([128, S], F32, tag="yc")
                    nc.scalar.copy(yc[:], ps[:])
                    nc.sync.dma_start(
                        y_ap[r0 + t * 128: r0 + (t + 1) * 128,
                             c * 512:(c + 1) * 512], yc[:])

    nc.compile()
    return nc


_NC_CACHE = {}


def _get_nc():
    if "nc" not in _NC_CACHE:
        nc = build_module()
        nc.m = get_hw_module(nc.m)
        _NC_CACHE["nc"] = nc
    return _NC_CACHE["nc"]


def kernel(hidden_states, Wq, Wk, Wv, Wo, bo):
    nc = _get_nc()
    hs = np.ascontiguousarray(np.asarray(hidden_states, dtype=np.float32))
    wq = np.ascontiguousarray(np.asarray(Wq, dtype=np.float32))
    wk = np.ascontiguousarray(np.asarray(Wk, dtype=np.float32))
    wv = np.ascontiguousarray(np.asarray(Wv, dtype=np.float32))
    wo = np.ascontiguousarray(np.asarray(Wo, dtype=np.float32))
    bon = np.ascontiguousarray(np.asarray(bo, dtype=np.float32))
    in_maps = [
        {
            "x": hs[c * BL:(c + 1) * BL].reshape(TOK, E),
            "wq": wq, "wk": wk, "wv": wv, "wo": wo, "bo": bon,
        }
        for c in range(N_CORES)
    ]
    res = run_bass_kernel_spmd(nc, in_maps, core_ids=list(range(N_CORES)))
    out = np.concatenate(
        [res.results[c]["y"].reshape(BL, S, E) for c in range(N_CORES)], axis=0)
    return out.astype(np.float32)


# revision 7
# speedup vs baseline: 1.2963x; 1.2963x over previous
"""Multi-headed self-attention (B=64, S=512, E=1024, H=16, causal, no 1/sqrt(d)
scale) as a Bass/Tile kernel for 8 Trainium2 NeuronCores.

Sharding: data-parallel over batch — each core processes 8 batches with
replicated weights; no collectives. Matmuls run in fp16 (projections, scores,
out-proj) / bf16 (attention*V, needed for exp() range) with fp32 PSUM
accumulation. Softmax skips the max-subtraction: scores for this problem are
bounded (|s| < 90 ⇒ exp() stays finite in fp32), and the denominators are
produced by augmenting V with a ones-column so the AV matmul emits them
directly.

Self-contained: hardcodes shapes; only needs concourse (on sys.path in the
container) + numpy.
"""

import numpy as np
from contextlib import ExitStack

import concourse.bass as bass
import concourse.tile as tile
from concourse import bacc, mybir
from concourse.bass_interp import get_hw_module
from concourse.bass_utils import run_bass_kernel_spmd
from concourse.masks import make_identity

F32 = mybir.dt.float32
F16 = mybir.dt.float16
BF16 = mybir.dt.bfloat16

B, S, E, H, D = 64, 512, 1024, 16, 64
N_CORES = 8
BL = B // N_CORES            # batches per core
TOK = BL * S                 # tokens per core
KE = E // 128                # 128-row tiles along e (8)
NT = S // 128                # 128-token tiles per batch (4)
NEG = -1.0e30


def build_module():
    nc = bacc.Bacc("TRN2", target_bir_lowering=False, debug=False,
                   num_devices=N_CORES)
    x_ap = nc.dram_tensor("x", [TOK, E], F32, kind="ExternalInput").ap()
    w_aps = {
        name: nc.dram_tensor(name, [E, E], F32, kind="ExternalInput").ap()
        for name in ("wq", "wk", "wv", "wo")
    }
    bo_ap = nc.dram_tensor("bo", [E], F32, kind="ExternalInput").ap()
    y_ap = nc.dram_tensor("y", [TOK, E], F32, kind="ExternalOutput").ap()

    with tile.TileContext(nc) as tc, ExitStack() as ctx:
        consts = ctx.enter_context(tc.tile_pool(name="consts", bufs=1))
        stage = ctx.enter_context(tc.tile_pool(name="stage", bufs=2))
        bigs = ctx.enter_context(tc.tile_pool(name="bigs", bufs=2))
        bigs1 = ctx.enter_context(tc.tile_pool(name="bigs1", bufs=1))
        ppool = ctx.enter_context(tc.tile_pool(name="ppool", bufs=2))
        ypool = ctx.enter_context(tc.tile_pool(name="ypool", bufs=2))
        small = ctx.enter_context(tc.tile_pool(name="small", bufs=8))
        psum = ctx.enter_context(tc.tile_pool(name="psum", bufs=4, space="PSUM"))
        psum_o = ctx.enter_context(tc.tile_pool(name="psum_o", bufs=4, space="PSUM"))

        # ---- constants ----
        ident = consts.tile([128, 128], F16)
        make_identity(nc, ident[:])
        ones_r = consts.tile([1, 128], F16)
        nc.vector.memset(ones_r[:], 1.0)
        bo_st = stage.tile([1, E], F32, tag="wstage")
        nc.sync.dma_start(bo_st[:], bo_ap[None, :])
        bo_sb = consts.tile([1, E], F16)
        nc.vector.tensor_copy(bo_sb[:], bo_st[:])

        w_sb = {}
        for name in ("wq", "wk", "wv", "wo"):
            wt = consts.tile([128, KE, E], F16, tag=name)
            for k in range(KE):
                ws = stage.tile([128, E], F32, tag="wstage")
                nc.sync.dma_start(ws[:], w_aps[name][k * 128:(k + 1) * 128, :])
                nc.vector.tensor_copy(wt[:, k, :], ws[:])
            w_sb[name] = wt

        # ---- per batch ----
        for b in range(BL):
            r0 = b * S
            # 1) load X, cast fp16, transpose via PE (identity matmul)
            xT = bigs.tile([128, KE, S], F16, tag="xT")   # [e, tok] fp16
            for t in range(NT):
                xs = stage.tile([128, E], F32, tag="xstage")
                nc.sync.dma_start(xs[:], x_ap[r0 + t * 128: r0 + (t + 1) * 128, :])
                xf = stage.tile([128, E], F16, tag="xf16")
                nc.vector.tensor_copy(xf[:], xs[:])
                for e in range(KE):
                    ptr = psum.tile([128, 128], F32, tag="mm512")
                    nc.tensor.matmul(ptr[:], lhsT=xf[:, e * 128:(e + 1) * 128],
                                     rhs=ident[:], start=True, stop=True)
                    nc.vector.tensor_copy(xT[:, e, t * 128:(t + 1) * 128], ptr[:])

            # 2) Q^T, K^T : [e_out, tok] fp16
            qT = bigs.tile([128, KE, S], F16, tag="qT")
            kT = bigs.tile([128, KE, S], F16, tag="kT")
            for eo in range(KE):
                for wname, dst in (("wq", qT), ("wk", kT)):
                    ps = psum.tile([128, S], F32, tag="mm512")
                    for k in range(KE):
                        nc.tensor.matmul(
                            ps[:], lhsT=w_sb[wname][:, k, eo * 128:(eo + 1) * 128],
                            rhs=xT[:, k, :], start=(k == 0), stop=(k == KE - 1))
                    nc.vector.tensor_copy(dst[:, eo, :], ps[:])

            # 3) V natural [tok, h, d] + ones column, bf16
            v_sb = bigs.tile([128, NT, H, D + 1], BF16, tag="v")
            for t in range(NT):
                for c in range(2):
                    ps = psum.tile([128, S], F32, tag="mm512")
                    for k in range(KE):
                        nc.tensor.matmul(
                            ps[:], lhsT=xT[:, k, t * 128:(t + 1) * 128],
                            rhs=w_sb["wv"][:, k, c * 512:(c + 1) * 512],
                            start=(k == 0), stop=(k == KE - 1))
                    nc.vector.tensor_copy(
                        v_sb[:, t, c * 8:(c + 1) * 8, 0:D],
                        ps[:].rearrange("p (h d) -> p h d", h=8))
                nc.vector.memset(v_sb[:, t, :, D:D + 1], 1.0)

            # 4) attention, head pairs (row-group packed scores), software
            #    pipelined: scores(j) emitted before AV(j-1).
            #    S^T[k, q] layout; P^T = exp(S^T) with the causal triangle of
            #    the diagonal block zeroed on GpSimd; AV emits O natural
            #    [q, d]+denominator per q-tile, normalized per-partition.
            oN = bigs1.tile([128, NT, KE, 128], F16, tag="oN")  # [tok, t, j, 2*d]

            def emit_scores(j):
                pts = ppool.tile([128, 2, NT, S], BF16, tag="pT")
                for i in range(NT):
                    w0 = i * 128
                    ps_a = psum.tile([128, S], F32, tag="mm512")
                    ps_b = psum.tile([128, S], F32, tag="mm512")
                    pss = [ps_a, ps_b]
                    for hp in range(2):
                        p0 = 64 * hp
                        nc.tensor.matmul(
                            pss[hp][:, w0:S],
                            lhsT=kT[p0:p0 + 64, j, w0:w0 + 128],
                            rhs=qT[p0:p0 + 64, j, w0:S], start=True, stop=True)
                    for hp in range(2):
                        nc.scalar.activation(
                            pts[:, hp, i, w0:S], pss[hp][:, w0:S],
                            mybir.ActivationFunctionType.Exp)
                        nc.gpsimd.affine_select(
                            out=pts[:, hp, i, w0:w0 + 128],
                            in_=pts[:, hp, i, w0:w0 + 128],
                            compare_op=mybir.AluOpType.is_ge, fill=0.0,
                            base=0, channel_multiplier=-1, pattern=[[1, 128]])
                return pts

            def emit_av(j, pts):
                for hp in range(2):
                    h = 2 * j + hp
                    for t in range(NT):
                        po = psum_o.tile([128, D + 1], F32, tag="po")
                        for i in range(t + 1):
                            nc.tensor.matmul(
                                po[:], lhsT=pts[:, hp, i, t * 128:(t + 1) * 128],
                                rhs=v_sb[:, i, h, :],
                                start=(i == 0), stop=(i == t))
                        linv = small.tile([128, 1], F32, tag="linv")
                        nc.vector.reciprocal(linv[:], po[:, D:D + 1])
                        nc.vector.tensor_scalar_mul(
                            oN[:, t, j, 64 * hp:64 * hp + 64], po[:, 0:D],
                            linv[:])

            prev = None
            for j in range(KE):
                pts = emit_scores(j)
                if prev is not None:
                    emit_av(j - 1, prev)
                prev = pts
            emit_av(KE - 1, prev)

            # transpose oN [tok, e] -> oT [e, tok] via PE identity matmuls
            oT = bigs.tile([128, KE, S], F16, tag="oT")
            for t in range(NT):
                for j in range(KE):
                    ptr = psum.tile([128, 128], F32, tag="mm512")
                    nc.tensor.matmul(ptr[:], lhsT=oN[:, t, j, :], rhs=ident[:],
                                     start=True, stop=True)
                    nc.scalar.copy(oT[:, j, t * 128:(t + 1) * 128], ptr[:])

            # 5) out-projection + bias, write y
            for t in range(NT):
                for c in range(2):
                    ps = psum.tile([128, S], F32, tag="mm512")
                    for k in range(KE):
                        nc.tensor.matmul(
                            ps[:], lhsT=oT[:, k, t * 128:(t + 1) * 128],
                            rhs=w_sb["wo"][:, k, c * 512:(c + 1) * 512],
                            start=(k == 0), stop=False)
                    nc.tensor.matmul(ps[:], lhsT=ones_r[:],
                                     rhs=bo_sb[:, c * 512:(c + 1) * 512],
                                     start=False, stop=True)
                    yc = ypool.tile([128, S], F32, tag="yc")
                    nc.scalar.copy(yc[:], ps[:])
                    nc.sync.dma_start(
                        y_ap[r0 + t * 128: r0 + (t + 1) * 128,
                             c * 512:(c + 1) * 512], yc[:])

    nc.compile()
    return nc


_NC_CACHE = {}


def _get_nc():
    if "nc" not in _NC_CACHE:
        nc = build_module()
        nc.m = get_hw_module(nc.m)
        _NC_CACHE["nc"] = nc
    return _NC_CACHE["nc"]


def kernel(hidden_states, Wq, Wk, Wv, Wo, bo):
    nc = _get_nc()
    hs = np.ascontiguousarray(np.asarray(hidden_states, dtype=np.float32))
    wq = np.ascontiguousarray(np.asarray(Wq, dtype=np.float32))
    wk = np.ascontiguousarray(np.asarray(Wk, dtype=np.float32))
    wv = np.ascontiguousarray(np.asarray(Wv, dtype=np.float32))
    wo = np.ascontiguousarray(np.asarray(Wo, dtype=np.float32))
    bon = np.ascontiguousarray(np.asarray(bo, dtype=np.float32))
    in_maps = [
        {
            "x": hs[c * BL:(c + 1) * BL].reshape(TOK, E),
            "wq": wq, "wk": wk, "wv": wv, "wo": wo, "bo": bon,
        }
        for c in range(N_CORES)
    ]
    res = run_bass_kernel_spmd(nc, in_maps, core_ids=list(range(N_CORES)))
    out = np.concatenate(
        [res.results[c]["y"].reshape(BL, S, E) for c in range(N_CORES)], axis=0)
    return out.astype(np.float32)


# revision 13
# speedup vs baseline: 1.3765x; 1.0618x over previous
"""Multi-headed self-attention (B=64, S=512, E=1024, H=16, causal, no 1/sqrt(d)
scale) as a Bass/Tile kernel for 8 Trainium2 NeuronCores.

Sharding: data-parallel over batch — each core processes 8 batches with
replicated weights; no collectives. Matmuls run in fp16 (projections, scores,
out-proj) / bf16 (attention*V, needed for exp() range) with fp32 PSUM
accumulation. Softmax skips the max-subtraction: scores for this problem are
bounded (|s| < 90 ⇒ exp() stays finite in fp32), and the denominators are
produced by augmenting V with a ones-column so the AV matmul emits them
directly.

Self-contained: hardcodes shapes; only needs concourse (on sys.path in the
container) + numpy.
"""

import numpy as np
from contextlib import ExitStack

import concourse.bass as bass
import concourse.tile as tile
from concourse import bacc, mybir
from concourse.bass_interp import get_hw_module
from concourse.bass_utils import run_bass_kernel_spmd
from concourse.masks import make_identity

F32 = mybir.dt.float32
F16 = mybir.dt.float16
BF16 = mybir.dt.bfloat16

B, S, E, H, D = 64, 512, 1024, 16, 64
N_CORES = 8
BL = B // N_CORES            # batches per core
TOK = BL * S                 # tokens per core
KE = E // 128                # 128-row tiles along e (8)
NT = S // 128                # 128-token tiles per batch (4)
NEG = -1.0e30


def build_module():
    nc = bacc.Bacc("TRN2", target_bir_lowering=False, debug=False,
                   num_devices=N_CORES)
    x_ap = nc.dram_tensor("x", [TOK, E], F32, kind="ExternalInput").ap()
    w_aps = {
        name: nc.dram_tensor(name, [E, E], F32, kind="ExternalInput").ap()
        for name in ("wq", "wk", "wv", "wo")
    }
    bo_ap = nc.dram_tensor("bo", [E], F32, kind="ExternalInput").ap()
    y_ap = nc.dram_tensor("y", [TOK, E], F32, kind="ExternalOutput").ap()

    with tile.TileContext(nc) as tc, ExitStack() as ctx:
        consts = ctx.enter_context(tc.tile_pool(name="consts", bufs=1))
        stage = ctx.enter_context(tc.tile_pool(name="stage", bufs=2))
        bigs = ctx.enter_context(tc.tile_pool(name="bigs", bufs=2))
        bigs1 = ctx.enter_context(tc.tile_pool(name="bigs1", bufs=1))
        ppool = ctx.enter_context(tc.tile_pool(name="ppool", bufs=2))
        ypool = ctx.enter_context(tc.tile_pool(name="ypool", bufs=2))
        small = ctx.enter_context(tc.tile_pool(name="small", bufs=3))
        psum = ctx.enter_context(tc.tile_pool(name="psum", bufs=4, space="PSUM"))
        psum_o = ctx.enter_context(tc.tile_pool(name="psum_o", bufs=4, space="PSUM"))

        # ---- constants ----
        ident = consts.tile([128, 128], F16)
        make_identity(nc, ident[:])
        ones_r = consts.tile([1, 128], F16)
        nc.vector.memset(ones_r[:], 1.0)
        bo_st = stage.tile([1, E], F32, tag="wstage")
        nc.sync.dma_start(bo_st[:], bo_ap[None, :])
        bo_sb = consts.tile([1, E], F16)
        nc.vector.tensor_copy(bo_sb[:], bo_st[:])

        w_sb = {}
        for name in ("wq", "wk", "wv", "wo"):
            wt = consts.tile([128, KE, E], F16, tag=name)
            for k in range(KE):
                ws = stage.tile([128, E], F32, tag="wstage")
                nc.sync.dma_start(ws[:], w_aps[name][k * 128:(k + 1) * 128, :])
                nc.vector.tensor_copy(wt[:, k, :], ws[:])
            w_sb[name] = wt

        # ---- per batch ----
        for b in range(BL):
            r0 = b * S
            # 1) load X, cast fp16, transpose via PE (identity matmul)
            xT = bigs.tile([128, KE, S], F16, tag="xT")   # [e, tok] fp16
            for t in range(NT):
                xs = stage.tile([128, E], F32, tag="xstage")
                nc.sync.dma_start(xs[:], x_ap[r0 + t * 128: r0 + (t + 1) * 128, :])
                xf = stage.tile([128, E], F16, tag="xf16")
                nc.vector.tensor_copy(xf[:], xs[:])
                for e in range(KE):
                    ptr = psum.tile([128, 128], F32, tag="mm512")
                    nc.tensor.matmul(ptr[:], lhsT=xf[:, e * 128:(e + 1) * 128],
                                     rhs=ident[:], start=True, stop=True)
                    nc.vector.tensor_copy(xT[:, e, t * 128:(t + 1) * 128], ptr[:])

            # 2) Q^T, K^T : [e_out, tok] fp16
            qT = bigs.tile([128, KE, S], F16, tag="qT")
            kT = bigs.tile([128, KE, S], F16, tag="kT")
            for eo in range(KE):
                for wname, dst in (("wq", qT), ("wk", kT)):
                    ps = psum.tile([128, S], F32, tag="mm512")
                    for k in range(KE):
                        nc.tensor.matmul(
                            ps[:], lhsT=w_sb[wname][:, k, eo * 128:(eo + 1) * 128],
                            rhs=xT[:, k, :], start=(k == 0), stop=(k == KE - 1))
                    nc.vector.tensor_copy(dst[:, eo, :], ps[:])

            # 3) V natural [tok, h, d] + ones column, bf16
            v_sb = bigs.tile([128, NT, H, D + 1], BF16, tag="v")
            for t in range(NT):
                for c in range(2):
                    ps = psum.tile([128, S], F32, tag="mm512")
                    for k in range(KE):
                        nc.tensor.matmul(
                            ps[:], lhsT=xT[:, k, t * 128:(t + 1) * 128],
                            rhs=w_sb["wv"][:, k, c * 512:(c + 1) * 512],
                            start=(k == 0), stop=(k == KE - 1))
                    nc.vector.tensor_copy(
                        v_sb[:, t, c * 8:(c + 1) * 8, 0:D],
                        ps[:].rearrange("p (h d) -> p h d", h=8))
                nc.vector.memset(v_sb[:, t, :, D:D + 1], 1.0)

            # 4) attention, head pairs (row-group packed scores), software
            #    pipelined: scores(j) emitted before AV(j-1).
            #    S^T[k, q] layout; P^T = exp(S^T) with the causal triangle of
            #    the diagonal block zeroed on GpSimd; AV = V_aug^T @ P^T gives
            #    O^T [d, q] + denominator row directly; normalize with
            #    fast-reciprocal + partition-broadcast of 1/l.
            oT = bigs.tile([128, KE, S], F16, tag="oT")

            def emit_scores(j):
                pts = ppool.tile([128, 2, NT, S], BF16, tag="pT")
                for i in range(NT):
                    w0 = i * 128
                    ps_a = psum.tile([128, S], F32, tag="mm512")
                    ps_b = psum.tile([128, S], F32, tag="mm512")
                    pss = [ps_a, ps_b]
                    for hp in range(2):
                        p0 = 64 * hp
                        nc.tensor.matmul(
                            pss[hp][:, w0:S],
                            lhsT=kT[p0:p0 + 64, j, w0:w0 + 128],
                            rhs=qT[p0:p0 + 64, j, w0:S], start=True, stop=True)
                    for hp in range(2):
                        nc.scalar.activation(
                            pts[:, hp, i, w0:S], pss[hp][:, w0:S],
                            mybir.ActivationFunctionType.Exp)
                        nc.gpsimd.affine_select(
                            out=pts[:, hp, i, w0:w0 + 128],
                            in_=pts[:, hp, i, w0:w0 + 128],
                            compare_op=mybir.AluOpType.is_ge, fill=0.0,
                            base=0, channel_multiplier=-1, pattern=[[1, 128]])
                return pts

            def emit_av(j, pts):
                for hp in range(2):
                    h = 2 * j + hp
                    p0 = 64 * hp
                    po = psum_o.tile([D + 1, S], F32, tag="po")
                    for i in range(NT):
                        w0 = i * 128
                        nc.tensor.matmul(
                            po[:, w0:S], lhsT=v_sb[:, i, h, :],
                            rhs=pts[:, hp, i, w0:S],
                            start=(i == 0), stop=(i == NT - 1))
                    lrow = small.tile([1, S], F32, tag="lrow")
                    nc.vector.tensor_copy(lrow[:], po[D:D + 1, :])
                    linv = small.tile([1, S], F32, tag="linv")
                    nc.vector.reciprocal_approx_fast(linv[:], lrow[:])
                    linb = small.tile([64, S], F32, tag="linb")
                    nc.gpsimd.partition_broadcast(linb[:], linv[:])
                    nc.vector.tensor_mul(oT[p0:p0 + 64, j, :], po[0:D, :],
                                         linb[:])

            prev = None
            for j in range(KE):
                pts = emit_scores(j)
                if prev is not None:
                    emit_av(j - 1, prev)
                prev = pts
            emit_av(KE - 1, prev)

            # 5) out-projection + bias, write y
            for t in range(NT):
                for c in range(2):
                    ps = psum.tile([128, S], F32, tag="mm512")
                    for k in range(KE):
                        nc.tensor.matmul(
                            ps[:], lhsT=oT[:, k, t * 128:(t + 1) * 128],
                            rhs=w_sb["wo"][:, k, c * 512:(c + 1) * 512],
                            start=(k == 0), stop=False)
                    nc.tensor.matmul(ps[:], lhsT=ones_r[:],
                                     rhs=bo_sb[:, c * 512:(c + 1) * 512],
                                     start=False, stop=True)
                    yc = ypool.tile([128, S], F32, tag="yc")
                    nc.scalar.copy(yc[:], ps[:])
                    nc.sync.dma_start(
                        y_ap[r0 + t * 128: r0 + (t + 1) * 128,
                             c * 512:(c + 1) * 512], yc[:])

    nc.compile()
    return nc


_NC_CACHE = {}


def _get_nc():
    if "nc" not in _NC_CACHE:
        nc = build_module()
        nc.m = get_hw_module(nc.m)
        _NC_CACHE["nc"] = nc
    return _NC_CACHE["nc"]


def kernel(hidden_states, Wq, Wk, Wv, Wo, bo):
    nc = _get_nc()
    hs = np.ascontiguousarray(np.asarray(hidden_states, dtype=np.float32))
    wq = np.ascontiguousarray(np.asarray(Wq, dtype=np.float32))
    wk = np.ascontiguousarray(np.asarray(Wk, dtype=np.float32))
    wv = np.ascontiguousarray(np.asarray(Wv, dtype=np.float32))
    wo = np.ascontiguousarray(np.asarray(Wo, dtype=np.float32))
    bon = np.ascontiguousarray(np.asarray(bo, dtype=np.float32))
    in_maps = [
        {
            "x": hs[c * BL:(c + 1) * BL].reshape(TOK, E),
            "wq": wq, "wk": wk, "wv": wv, "wo": wo, "bo": bon,
        }
        for c in range(N_CORES)
    ]
    res = run_bass_kernel_spmd(nc, in_maps, core_ids=list(range(N_CORES)))
    out = np.concatenate(
        [res.results[c]["y"].reshape(BL, S, E) for c in range(N_CORES)], axis=0)
    return out.astype(np.float32)
